# revision 22
# baseline (speedup 1.0000x reference)
"""Conv2d 3x3 (pad 1, stride 1) + bias on 8 Trainium2 cores.

Problem: x [32,128,56,56] f32, weights [256,128,3,3] f32, bias [256] f32
         -> out [32,256,56,56] f32.

Strategy
--------
Data-parallel over batch (4 images/core) + 1D Winograd F(2,3) along W.

For each output pair (2u, 2u+1) and each vertical tap kh, the 3-tap
horizontal conv costs 4 multiplies instead of 6: with d0..d3 the 4
padded inputs around the pair,
  t0 = d0-d2, t1 = d1+d2, t2 = d2-d1, t3 = d1-d3
  m_p = sum_cin sum_kh w'_p[kh] * t_p[row r+kh]
        (w'_0=g0, w'_1=(g0+g1+g2)/2, w'_2=(g0-g1+g2)/2, w'_3=g2)
  out[2u]   = m0+m1+m2+bias
  out[2u+1] = m1-m2-m3+bias
PE work drops from 9 to 6 matmul-columns per output pixel (and the
junk 57-stride column of a direct kernel disappears): 150.5K cols/core
= 62.7us at 2.4GHz vs 95.8us direct.

Layout: the host splits the padded rows into four flat shifted planes
eL/eR/oL/oR (even/odd columns, left/right-shifted, [58*28] flat) so
each t-plane is ONE contiguous packed-bf16 tensor_tensor on DVE.
Strided 28-element-run transforms measured 2-4x slower on HW than
their cost-model estimate; flat packed ops hit the true 2x DVE mode.
Each plane is a separate contiguous DMA (the tile dep tracker uses
bounding intervals, so interleaved-chunk writes create false deps).
The t-planes [cin, 58*28] use flat row-stride 28: vertical tap kh of a
group at flat col lo is the constant offset lo + kh*28, so matmuls run
seamlessly across row boundaries.

Work unit = a 784-pair-col double-group: each m_p accumulates in a
2-bank PSUM tile via 6 matmuls (3 kh x [0:512]+[512:784] bank-aligned
splits); 4 m-tiles = all 8 banks. There is no group-level double
buffering - instead each PSUM bank has exactly ONE drain reader, so
the next double-group's first matmuls only wait on a drain that
completed mid-previous-group. Wide drains halve per-op overheads vs
392-col groups and leave every drain engine at <=80% of the PE window:
  ACT:    a1 = Ident(m1+bias), a2 = Copy(m2), a3 = Copy(-m3)
  DVE:    w1 = a1-a2 (sbuf 2x), u0 = m0+a1 (the one psum op), out0 = u0+a2
  GpSimd: out1 = w1+a3 (sbuf only; GpSimd cannot read PSUM)
The two cout-halves interleave per double-group (dg0.h0, dg0.h1,
dg1.h0, ...), which doubles the compute runway per input chunk at
startup. Outputs stay as separate even/odd bf16 planes (the host
interleaves and widens to f32; tolerance is 2e-2, bf16 out lands
~7e-3), one DMA per double-group.

Startup: warmup matmuls ramp the PE clock while the first DMA wave
lands (row-tier 1 of the four input planes, bias, weights in first-use
order). Transfers not needed before ~+2.5us (row tiers 2-3, half-1
weight tail, image prefetches) are gated behind warmup/ACT WAW touches
so the Tile scheduler cannot hoist them into the critical wave. Image
b+1's planes prefetch as four per-plane DMAs in use order and its
t-planes build on DVE one per group slot, each landing just before
first use; plane 3 builds in image b+1's own first slot (pre-matmul).
The final half tapers (784,512,272) and its last drain chain avoids
the GpSimd queue and ships its two output planes on separate queues.
"""

import numpy as np
import ml_dtypes

import concourse.bacc as bacc
import concourse.mybir as mybir
import concourse.tile as tile
from concourse.bass_utils import run_bass_kernel_spmd

B, CIN, H, W = 32, 128, 56, 56
COUT = 256
NCORES = 8
BLOC = B // NCORES  # images per core
NR = H + 2  # 58 padded rows
PC = W // 2  # 28 output pairs per row
PLANE = NR * PC  # 1624 flat plane cols
NPAIR = H * PC  # 1568 output pair-cols per image-half
NWARM = 4

# Weight stationary order per half = first-use order: m1, m2, m0, m3.
WORDER = [1, 2, 0, 3]
PIDX = {p: i for i, p in enumerate(WORDER)}
# Source-plane pairs per t-plane: t_p = QSRC[p][0] (op) QSRC[p][1],
# with planes [eL, eR, oL, oR] = indices 0..3.
QSRC = {0: (0, 1), 1: (2, 1), 2: (1, 2), 3: (2, 3)}  # (in0, in1)
QNEED = {0: 3, 1: 2, 2: 2, 3: 4}  # prefetch planes needed (see QORDER)
QORDER = [2, 1, 0, 3]  # prefetch DMA order: oL, eR, eL, oR

NORM_GROUPS = [(0, 784), (784, 784)]
# Image 0: start-taper so the first matmuls need only row tier 1.
FIRST_GROUPS = [(0, 272), (272, 512), (784, 512), (1296, 272)]
# Last image: end-taper to shorten the final drain + DMA chain.
LAST_GROUPS = [(0, 784), (784, 512), (1296, 272)]
# Image 0 input row tiers (per-plane DMA + transform granularity).
# FIRST_GROUPS dg_i needs plane rows < 12/30/49/58.
TIERS0 = [(0, 16), (16, 34), (34, 58)]

_nc_cache = None


def _build():
    f32 = mybir.dt.float32
    bf16 = mybir.dt.bfloat16
    COPY = mybir.ActivationFunctionType.Copy
    IDENT = mybir.ActivationFunctionType.Identity
    nc = bacc.Bacc("TRN2", target_bir_lowering=False)
    x_d = nc.dram_tensor("xq", [BLOC, CIN, 4, PLANE], bf16, kind="ExternalInput")
    w_d = nc.dram_tensor("wT", [CIN, 2 * 4 * 3 * 128], bf16, kind="ExternalInput")
    b_d = nc.dram_tensor("bias2", [128, 2], f32, kind="ExternalInput")
    o_d = nc.dram_tensor("out", [BLOC, 2, 128, 2, NPAIR], bf16, kind="ExternalOutput")

    def wcol(h, p, kh):
        return ((h * 4 + PIDX[p]) * 3 + kh) * 128

    with tile.TileContext(nc) as tc:
        with (
            tc.tile_pool(name="wpool", bufs=1) as wpool,
            tc.tile_pool(name="xpool", bufs=2) as xpool,
            tc.tile_pool(name="tpool", bufs=8) as tpool,
            tc.tile_pool(name="upool", bufs=3) as upool,
            tc.tile_pool(name="vpool", bufs=3) as vpool,
            tc.tile_pool(name="opool", bufs=3) as opool,
            tc.tile_pool(name="psum", bufs=4, space="PSUM") as psum,
        ):
            wsb = wpool.tile([CIN, 2 * 4 * 3 * 128], bf16)
            bsb = wpool.tile([128, 2], f32)
            wub = wpool.tile([128, 512], bf16)
            dmy = wpool.tile([128, 2], bf16)
            nc.vector.memset(wub[:], 0.0)
            # Dummy Identity activation: pulls the ~1.3us activation
            # table load to the front of the ACT queue (its engine
            # queue depth is 0, so a late table load would stall it).
            nc.scalar.activation(dmy[:], wub[:, :2], IDENT)

            xqs = [xpool.tile([CIN, 4, PLANE], bf16, tag="xq", name="xq0")]
            tpls = [
                [
                    tpool.tile([CIN, PLANE], bf16, tag="tp", name=f"tp0_{p}")
                    for p in range(4)
                ]
            ]

            # PE warmup: matmul 1 issues as soon as the memset lands
            # and its completion ungates the non-critical DMAs below;
            # 2-4 keep the clock ramping while the first wave lands.
            wup = psum.tile([128, 512], f32, tag="pt", name="wup")
            nc.tensor.matmul(
                wup[:], lhsT=wub[:, :128], rhs=wub[:], start=True, stop=True
            )
            # WAW touches: gate row tiers 2-3 behind warmup matmul 1
            # (the Tile scheduler hoists dep-free DMAs past queue
            # order, so position alone cannot keep them out of the
            # critical first wave).
            xq0 = xqs[0]
            for (r0, r1) in TIERS0[1:]:
                for q in range(4):
                    nc.vector.tensor_scalar_mul(
                        xq0[:, q, r0 * PC : r0 * PC + 2], wup[:, :2], 0.0
                    )
            for _ in range(NWARM - 1):
                nc.tensor.matmul(
                    wup[:], lhsT=wub[:, :128], rhs=wub[:], start=True, stop=True
                )

            # Startup DMA wave, ordered by first-use deadline. SP: the
            # four tier-1 plane chunks (use order oL,eR,eL,oR), then
            # gated tiers 2-3. ACT: bias, weights in first-use order.
            t1_, t2_, t3_ = TIERS0
            for q in QORDER:
                nc.sync.dma_start(
                    xq0[:, q, t1_[0] * PC : t1_[1] * PC],
                    x_d[0, :, q, t1_[0] * PC : t1_[1] * PC],
                )
            nc.scalar.dma_start(bsb[:], b_d[:])
            nc.scalar.dma_start(wsb[:, 0:1536], w_d[:, 0:1536])
            nc.scalar.dma_start(wsb[:, 1536:2304], w_d[:, 1536:2304])
            nc.scalar.dma_start(wsb[:, 2304:3072], w_d[:, 2304:3072])
            for (r0, r1) in TIERS0[1:]:
                for q in QORDER:
                    nc.sync.dma_start(
                        xq0[:, q, r0 * PC : r1 * PC],
                        x_d[0, :, q, r0 * PC : r1 * PC],
                    )

            def transform(bi, r0, r1, only=None):
                """t-plane rows [r0,r1) for image slot bi (DVE, flat)."""
                xq = xqs[bi]
                tp = tpls[bi]
                lo, hi = r0 * PC, r1 * PC
                order = [only] if only is not None else WORDER
                for p in order:
                    i0, i1 = QSRC[p]
                    fn = nc.vector.tensor_add if p == 1 else nc.vector.tensor_sub
                    fn(tp[p][:, lo:hi], xq[:, i0, lo:hi], xq[:, i1, lo:hi])

            transform(0, *TIERS0[0])

            def do_group(b, h, lo, n, last_group=False, pre_drains=None,
                         pre_mm=None, prefetch=False):
                if pre_mm is not None:
                    pre_mm()
                tp = tpls[b]
                ranges = [(0, min(n, 512))] + ([(512, n)] if n > 512 else [])
                pts = {}
                for p in WORDER:
                    pts[p] = psum.tile(
                        [128, 784], f32, tag="pt", name=f"pt_b{b}h{h}l{lo}p{p}"
                    )
                    for kh in range(3):
                        c = wcol(h, p, kh)
                        for (r0, r1) in ranges:
                            nc.tensor.matmul(
                                pts[p][:, r0:r1],
                                lhsT=wsb[:, c : c + 128],
                                rhs=tp[p][:, lo + kh * PC + r0 : lo + kh * PC + r1],
                                start=(kh == 0),
                                stop=(kh == 2),
                            )
                a1 = vpool.tile([128, 784], bf16, tag="a1")
                a2 = vpool.tile([128, 784], bf16, tag="a2")
                a3 = vpool.tile([128, 784], bf16, tag="a3")
                u0 = upool.tile([128, 784], bf16, tag="u0")
                w1 = upool.tile([128, 784], bf16, tag="w1")
                ot = opool.tile([128, 2, 784], bf16, tag="ot")
                bvec = bsb[:, h : h + 1]
                nc.scalar.activation(a1[:, :n], pts[1][:, :n], IDENT, bias=bvec)
                nc.scalar.activation(a2[:, :n], pts[2][:, :n], COPY)
                nc.scalar.activation(a3[:, :n], pts[3][:, :n], COPY, scale=-1.0)
                if prefetch:
                    # Prefetch next image's four planes in use order,
                    # each gated behind this group's a1 (an early
                    # 1.7MB prefetch would starve the transfers gating
                    # the PE; per-plane DMAs let each t-plane build as
                    # soon as its sources land).
                    xqn = xpool.tile(
                        [CIN, 4, PLANE], bf16, tag="xq", name=f"xq{b+1}"
                    )
                    xqs.append(xqn)
                    tpls.append(
                        [
                            tpool.tile(
                                [CIN, PLANE], bf16, tag="tp", name=f"tp{b+1}_{p}"
                            )
                            for p in range(4)
                        ]
                    )
                    for q in QORDER:
                        nc.gpsimd.tensor_scalar_mul(
                            xqn[:, q, 0:2], a1[:, 0:2], 0.0
                        )
                        nc.sync.dma_start(xqn[:, q], x_d[b + 1, :, q])
                if pre_drains is not None:
                    pre_drains()
                # out0 = (m0 + a1) + a2 ; out1 = (a1 - a2) + a3
                nc.vector.tensor_sub(w1[:, :n], a1[:, :n], a2[:, :n])
                nc.vector.tensor_add(u0[:, :n], pts[0][:, :n], a1[:, :n])
                nc.vector.tensor_add(ot[:, 0, :n], u0[:, :n], a2[:, :n])
                out1_eng = nc.vector if last_group else nc.gpsimd
                out1_eng.tensor_add(ot[:, 1, :n], w1[:, :n], a3[:, :n])
                if last_group:
                    nc.sync.dma_start(o_d[b, h, :, 0, lo : lo + n], ot[:, 0, :n])
                    nc.scalar.dma_start(o_d[b, h, :, 1, lo : lo + n], ot[:, 1, :n])
                else:
                    nc.sync.dma_start(o_d[b, h, :, :, lo : lo + n], ot[:, :, :n])

            # Transform jobs per (image, slot): slot = dg_index*2 + h.
            # Image 0's early slots carry its row tiers 2-3; planes
            # 1/2/0 for image b+1 build in image b's later slots after
            # the prefetch planes land; plane 3 builds in image b+1's
            # own slot 0, before its matmuls.
            def whole(bi, p):
                return lambda bi=bi, p=p: transform(bi, 0, NR, only=p)

            def tier(ti):
                return lambda ti=ti: transform(0, *TIERS0[ti])

            jobs = {b: {} for b in range(BLOC)}
            jobs[0][1] = [tier(1)]
            jobs[0][3] = [tier(2)]
            for b in range(BLOC):
                if b > 0:
                    jobs[b].setdefault(0, []).append(whole(b, 3))
                if b + 1 < BLOC:
                    base = 5 if b == 0 else 1
                    jobs[b][base] = [whole(b + 1, 1)]
                    jobs[b][base + 1] = [whole(b + 1, 2)]
                    jobs[b][base + 2] = [whole(b + 1, 0)]

            for b in range(BLOC):
                if b == 0:
                    groups = FIRST_GROUPS
                elif b == BLOC - 1:
                    groups = LAST_GROUPS
                else:
                    groups = NORM_GROUPS
                for dgi, (lo, n) in enumerate(groups):
                    for h in range(2):
                        slot = dgi * 2 + h
                        jl = jobs[b].get(slot)
                        pre = (
                            None if not jl
                            else (lambda jl=jl: [f() for f in jl])
                        )
                        # Slot-0 jobs build a plane this group's own
                        # matmuls read - emit them BEFORE the matmuls
                        # (dep tracking is program-order based).
                        do_group(
                            b, h, lo, n,
                            last_group=(
                                b == BLOC - 1
                                and dgi == len(groups) - 1
                                and h == 1
                            ),
                            pre_drains=pre if slot != 0 else None,
                            pre_mm=pre if slot == 0 else None,
                            prefetch=(
                                dgi == 0 and h == 0 and b + 1 < BLOC
                            ),
                        )

    nc.compile()
    return nc


def _get_nc():
    global _nc_cache
    if _nc_cache is None:
        _nc_cache = _build()
    return _nc_cache


def _prep_inputs(x, weights, bias):
    x = np.asarray(x, dtype=np.float32)
    weights = np.asarray(weights, dtype=np.float32)
    bias = np.ascontiguousarray(np.asarray(bias, dtype=np.float32))

    xb = x.astype(ml_dtypes.bfloat16)
    xpad = np.pad(xb, ((0, 0), (0, 0), (1, 1), (1, 1)))  # [B,C,58,58]
    xe = xpad[:, :, :, 0::2]  # [B,C,58,29]
    xo = xpad[:, :, :, 1::2]
    planes = [
        xe[:, :, :, 0:PC],      # eL
        xe[:, :, :, 1 : PC + 1],  # eR
        xo[:, :, :, 0:PC],      # oL
        xo[:, :, :, 1 : PC + 1],  # oR
    ]
    xq = np.ascontiguousarray(
        np.stack([p.reshape(B, CIN, PLANE) for p in planes], axis=2)
    )  # [B, C, 4, 1624]

    g = weights.reshape(2, 128, CIN, 3, 3)  # [h, co, cin, kh, kw]
    w0 = g[..., 0]
    w1 = (g[..., 0] + g[..., 1] + g[..., 2]) * 0.5
    w2 = (g[..., 0] - g[..., 1] + g[..., 2]) * 0.5
    w3 = g[..., 2]
    wlist = [w0, w1, w2, w3]
    # stack in WORDER; axes [h, p, co, cin, kh] -> [cin, h, p, kh, co]
    wp = np.stack([wlist[p] for p in WORDER], axis=1)
    wT = np.ascontiguousarray(wp.transpose(3, 0, 1, 4, 2)).reshape(
        CIN, 2 * 4 * 3 * 128
    ).astype(ml_dtypes.bfloat16)
    b2 = np.ascontiguousarray(bias.reshape(2, 128).T)  # b2[p,h] = bias[h*128+p]

    return [
        {
            "xq": np.ascontiguousarray(xq[i * BLOC : (i + 1) * BLOC]),
            "wT": wT,
            "bias2": b2,
        }
        for i in range(NCORES)
    ]


def _run(inputs, trace=False):
    in_maps = _prep_inputs(inputs["x"], inputs["weights"], inputs["bias"])
    res = run_bass_kernel_spmd(
        _get_nc(), in_maps, core_ids=list(range(NCORES)), trace=trace
    )
    o = np.concatenate([np.asarray(r["out"]) for r in res.results], axis=0)
    # [B, 2h, 128co, 2pl, 1568] bf16 -> [B, 256, 56, 56] f32
    o = o.astype(np.float32).reshape(B, 2, 128, 2, H, PC)
    o = o.transpose(0, 1, 2, 4, 5, 3).reshape(B, COUT, H, W)
    return np.ascontiguousarray(o), res


def kernel(x, weights, bias):
    out, _ = _run({"x": x, "weights": weights, "bias": bias})
    return out
